# revision 31
# baseline (speedup 1.0000x reference)
"""Trainium2 Bass kernel for a bilinear critic:

    xe = relu(x @ Wx1 + bx1) @ Wx2 + bx2          # [B, 32]
    ye = relu(y @ Wy1 + by1) @ Wy2 + by2          # [B, 32]
    scores = (xe @ W_bil) @ ye.T + b_bil[0]       # [B, B]
    returns (scores, 0.0)

B=8192, D_IN=64, D_HID=256, D_EMB=32. 8 NeuronCores, data-parallel over
rows of x (1024 rows/core); the y embedding is computed redundantly on
every core (the MLP is tiny next to the 256 MiB output write, which is
the roofline term). Everything on-chip is kept in TRANSPOSED layout
([feature, batch]) so the contraction dim lands on SBUF partitions with
no on-chip transposes; the host feeds x.T / y.T.

fp32 matmuls run at 1/4 PE rate, so all big GEMMs use an exact 3-term
bf16 hi/lo split (a = ah + al; a@b ~ ah@bh + al@bh + ah@bl, error
~2^-18): the PE streams bf16 at full rate and accumulates in fp32 PSUM.
Splits of inputs/weights are host-precomputed; h's split is computed
on-chip (ACT relu -> hh, DVE (max(psum,0) - hh) -> hl).

Bias folding:
  - by1 enters the layer-1 matmul as two extra contraction rows
    (bf16 hi/lo) against ones-rows of the input pack (exact, and keeps
    relu after the full affine).
  - W_bil and by2 are folded on host into the y-side layer-2:
    ywe = ye @ W_bil^T, so scores = xe_aug^T @ ywe_aug.
  - b_bil enters the scores matmul as two extra hi/lo rows against
    ones-rows on the y side.

Main-loop structure: column chunks of 512 are processed in PAIRS so the
score copies and stores move [128, 1024] at a time (amortized op
overhead; 4 KB contiguous DMA descriptors) and the stationary xst
weights are loaded once per row-block for both chunk matmuls.
"""

import numpy as np
from contextlib import ExitStack

import ml_dtypes
import concourse.bass as bass
import concourse.bacc as bacc
import concourse.tile as tile
from concourse import mybir
from concourse.bass_utils import run_bass_kernel_spmd

B, D_IN, D_HID, D_EMB = 8192, 64, 256, 32
N_CORES = 8
R = B // N_CORES            # 1024 rows of x per core
CH = 512                    # column-chunk width (one PSUM bank of fp32)
NCH = B // CH               # 16 chunks over y's batch dim
RCH = R // CH               # 2 chunks over the core's x rows
KS = 3 * D_EMB + 2          # 98: scores stacked contraction
F32 = mybir.dt.float32
BF16 = mybir.dt.bfloat16
NPBF = ml_dtypes.bfloat16

Relu = mybir.ActivationFunctionType.Relu
Ident = mybir.ActivationFunctionType.Identity
ADD = mybir.AluOpType.add
SUB = mybir.AluOpType.subtract
MAX = mybir.AluOpType.max

TRACE = False               # test.py flips this to profile
LAST_RESULT = None          # BassKernelResults of the last run

_NC = None


def _emit(ctx, tc, t):
    nc = tc.nc
    PS = bass.MemorySpace.PSUM

    consts = ctx.enter_context(tc.tile_pool(name="consts", bufs=1))
    hpool = ctx.enter_context(tc.tile_pool(name="h", bufs=3))
    yepool = ctx.enter_context(tc.tile_pool(name="ye", bufs=4))
    embp = ctx.enter_context(tc.tile_pool(name="emb", bufs=1))
    outp = ctx.enter_context(tc.tile_pool(name="outs", bufs=8))
    # embed psum: layer-1 [128,512] and layer-2 [32,512] share 2 slots
    # (they are live at different pipeline stages); scores get 3x2 banks.
    ps_m = ctx.enter_context(tc.tile_pool(name="ps_m", bufs=2, space=PS))
    ps_s = ctx.enter_context(tc.tile_pool(name="ps_s", bufs=3, space=PS))

    # One packed bf16 const blob + one packed f32 bias blob: each DMA
    # instruction costs ~600ns of DGE descriptor emission, so the many
    # small weight/bias loads are consolidated into 2 instructions.
    cb = consts.tile([128, 1280], BF16, tag="cblob")
    nc.sync.dma_start(cb[:], t["cblob"][:])
    fb = consts.tile([D_EMB, 3], F32, tag="fblob")
    nc.sync.dma_start(fb[:], t["fblob"][:])
    w2x = [cb[:, i * 32:(i + 1) * 32] for i in range(4)]
    w2y = [cb[:, 128 + i * 32:128 + (i + 1) * 32] for i in range(4)]
    w1x_a = cb[:, 256:512]          # [w1h; w1l] stacked rows
    w1x_b = cb[0:66, 512:768]       # [w1h; b1h; b1l]
    w1y_a = cb[:, 768:1024]
    w1y_b = cb[0:66, 1024:1280]
    bx2_sb = fb[:, 0:1]
    by2_sb = fb[:, 1:2]
    bb_sb = fb[0:2, 2:3]
    # inputs, host-packed [128, 2n]: per 512-chunk a [128, 1024] block
    # laid out as [[ah; ah] | [al; 1; 1; pad]] pairing with (w1a, w1b)
    x_in = consts.tile([128, 2 * R], BF16, tag="x_in")
    nc.sync.dma_start(x_in[:], t["xts2"][:])
    # whole packed y prefetched into SBUF upfront (3.2 MB) in 4 big
    # DMAs; no per-chunk loads -> stores own the DMA queues.
    y_in = consts.tile([128, 2 * B], BF16, tag="y_in")
    for q in range(4):
        qs = slice(q * B // 2, (q + 1) * B // 2)
        nc.sync.dma_start(y_in[:, qs], t["yts2"][:, qs])

    xst = embp.tile([KS, R], BF16, tag="xst")   # [xh; xl; xh; bh; bl]
    zero2 = consts.tile([2, R], BF16, tag="zero2")
    nc.gpsimd.memset(zero2[:], 0.0)

    def l1_matmuls(ph, w1a, w1b, src, nb, msl):
        """h_pre[128, ch] = w1h.ah + w1l.ah + w1h.al + b1h + b1l."""
        sa = src[:, 2 * nb * CH:(2 * nb + 1) * CH]
        sb = src[0:66, (2 * nb + 1) * CH:(2 * nb + 2) * CH]
        nc.tensor.matmul(ph[:], w1a[:, msl], sa, start=True, stop=False)
        nc.tensor.matmul(ph[:], w1b[:, msl], sb, start=False, stop=True)

    def l2_matmuls(pe_, w2, hh, hl, cs=None):
        """e[32, ch] += w2h.hh + w2l.hh + w2h.hl (6 accumulating mms)."""
        pairs = [(w2[0], hh[0]), (w2[1], hh[1]), (w2[2], hh[0]),
                 (w2[3], hh[1]), (w2[0], hl[0]), (w2[1], hl[1])]
        for i, (w, h) in enumerate(pairs):
            rhs = h[:] if cs is None else h[:, cs]
            nc.tensor.matmul(pe_[:], w[:], rhs,
                             start=(i == 0), stop=(i == len(pairs) - 1))

    # ---- x embedding (8 row-blocks of this core's 1024 rows)
    hhx = [[None, None] for _ in range(RCH)]
    hlx = [[None, None] for _ in range(RCH)]
    for nb in range(RCH):
        for mb in range(2):
            msl = slice(mb * 128, (mb + 1) * 128)
            ph = ps_m.tile([128, CH], F32, tag="pm")
            l1_matmuls(ph, w1x_a, w1x_b, x_in, nb, msl)
            hh = embp.tile([128, CH], BF16, tag=f"hhx{nb}{mb}")
            nc.scalar.activation(hh[:], ph[:], Relu)
            hl = embp.tile([128, CH], BF16, tag=f"hlx{nb}{mb}")
            nc.vector.scalar_tensor_tensor(hl[:], ph[:], 0.0, hh[:], MAX, SUB)
            hhx[nb][mb] = hh
            hlx[nb][mb] = hl

    for nb in range(RCH):
        cs = slice(nb * CH, (nb + 1) * CH)
        pex = ps_m.tile([D_EMB, CH], F32, tag="pm")
        l2_matmuls(pex, w2x, hhx[nb], hlx[nb])
        nc.scalar.activation(xst[0:32, cs], pex[:], Ident, bias=bx2_sb[:])
        nc.vector.scalar_tensor_tensor(
            xst[32:64, cs], pex[:], bx2_sb[:], xst[0:32, cs], ADD, SUB)
        nc.vector.tensor_copy(xst[64:96, cs], xst[0:32, cs])
    # scores-bias rows: xst[96] = bf16_hi(b_bil), xst[97] = bf16_lo(b_bil)
    nc.scalar.activation(xst[96:98, :], zero2[:], Ident, bias=bb_sb[:])

    def y_embed(nb):
        """-> yst tile [98, 512] = [yh; yh; yl; 1; 1] for column chunk nb."""
        hhy, hly = [], []
        for mb in range(2):
            msl = slice(mb * 128, (mb + 1) * 128)
            ph = ps_m.tile([128, CH], F32, tag="pm")
            l1_matmuls(ph, w1y_a, w1y_b, y_in, nb, msl)
            hh = hpool.tile([128, CH], BF16, tag=f"hhy{mb}")
            nc.scalar.activation(hh[:], ph[:], Relu)
            hl = hpool.tile([128, CH], BF16, tag=f"hly{mb}")
            nc.vector.scalar_tensor_tensor(hl[:], ph[:], 0.0, hh[:], MAX, SUB)
            hhy.append(hh)
            hly.append(hl)
        pey = ps_m.tile([D_EMB, CH], F32, tag="pm")
        l2_matmuls(pey, w2y, hhy, hly)
        # xst is [xh; xl; xh; ...], so yst pairs as [yh; yh; yl; 1; 1]
        yst = yepool.tile([KS, CH], BF16, tag="yst")
        nc.scalar.activation(yst[0:32, :], pey[:], Ident, bias=by2_sb[:])
        nc.vector.tensor_copy(yst[32:64, :], yst[0:32, :])
        nc.vector.scalar_tensor_tensor(
            yst[64:96, :], pey[:], by2_sb[:], yst[0:32, :], ADD, SUB)
        nc.gpsimd.memset(yst[96:98, :], 1.0)
        return yst

    # ---- y embedding + scores, chunk PAIRS (1024 output cols at a time)
    for pr in range(NCH // 2):
        nbA, nbB = 2 * pr, 2 * pr + 1
        ystA = y_embed(nbA)
        ystB = y_embed(nbB)
        for mb in range(8):  # row-blocks of 128 within this core's R rows
            ps = ps_s.tile([128, 2 * CH], F32, tag="ps")
            lhs = xst[:, mb * 128:(mb + 1) * 128]
            nc.tensor.matmul(ps[:, 0:CH], lhs, ystA[:], start=True, stop=True)
            nc.tensor.matmul(ps[:, CH:2 * CH], lhs, ystB[:],
                             start=True, stop=True)
            ot = outp.tile([128, 2 * CH], F32, tag="ot")
            # 5:3 ACT:DVE split (DVE also carries the hl/yl arithmetic)
            if mb in (0, 2, 4, 5, 7):
                nc.scalar.activation(ot[:], ps[:], Ident)
            else:
                nc.vector.tensor_copy(ot[:], ps[:])
            nc.sync.dma_start(
                t["scores"][mb * 128:(mb + 1) * 128,
                            nbA * CH:(nbA + 2) * CH], ot[:])


def _build():
    nc = bacc.Bacc(
        "TRN2", target_bir_lowering=False, debug=False, num_devices=N_CORES
    )
    t = {}

    def din(name, shape, dt):
        t[name] = nc.dram_tensor(name, shape, dt, kind="ExternalInput").ap()

    din("xts2", [128, 2 * R], BF16)
    din("yts2", [128, 2 * B], BF16)
    din("cblob", [128, 1280], BF16)
    din("fblob", [D_EMB, 3], F32)
    t["scores"] = nc.dram_tensor("scores", [R, B], F32, kind="ExternalOutput").ap()

    with tile.TileContext(nc) as tc:
        with ExitStack() as ctx:
            _emit(ctx, tc, t)
    nc.compile()
    return nc


def _split(a):
    """f32 array -> (hi, lo) bf16 arrays with a ~= hi + lo (err ~2^-18)."""
    hi = a.astype(NPBF)
    lo = (a - hi.astype(np.float32)).astype(NPBF)
    return hi, lo


def _stack1(w1, b1):
    """Layer-1 weight stacks: a=[w1h; w1l] [128,256], b=[w1h; b1h; b1l] [66,256]."""
    wh, wl = _split(w1)                      # [64, 256]
    bh, bl = _split(b1.reshape(1, -1))       # [1, 256]
    a = np.concatenate([wh, wl], axis=0)
    b = np.concatenate([wh, bh, bl], axis=0)
    return a, b


def _packin(aT):
    """Input pack [128, 2n]: per 512-chunk a [128,1024] block
    [[ah; ah] | [al; 1; 1; 0pad]] pairing with (_stack1.a, _stack1.b)."""
    ah, al = _split(aT)                      # [64, n]
    n = aT.shape[1]
    out = np.zeros((128, 2 * n), NPBF)
    for c in range(n // CH):
        cs = slice(c * CH, (c + 1) * CH)
        blk = slice(2 * c * CH, (2 * c + 1) * CH)
        out[0:64, blk] = ah[:, cs]
        out[64:128, blk] = ah[:, cs]
        blk2 = slice((2 * c + 1) * CH, (2 * c + 2) * CH)
        out[0:64, blk2] = al[:, cs]
        out[64:66, blk2] = np.ones((2, CH), NPBF)
    return out


def kernel(**inputs):
    global _NC, LAST_RESULT
    f = lambda k: np.ascontiguousarray(np.asarray(inputs[k], dtype=np.float32))

    x, y = f("x"), f("y")
    xts_full = _packin(np.ascontiguousarray(x.T))    # [128, 2*8192]
    yts2 = _packin(np.ascontiguousarray(y.T))        # [128, 2*8192]

    # fold W_bil (and by2) into the y-side layer 2: ywe = ye @ W_bil^T
    Wb = f("W_bil").astype(np.float64)
    wy2f = (f("Wy2").astype(np.float64) @ Wb.T).astype(np.float32)
    by2f = (f("by2").astype(np.float64) @ Wb.T).astype(np.float32)

    def stack2cols(w2):
        """[256,32] -> [128, 128]: 4 side-by-side K-chunks of [w2h; w2l]."""
        wh, wl = _split(w2)
        s = np.concatenate([wh, wl], axis=0)     # [512, 32]
        return np.concatenate(
            [s[i * 128:(i + 1) * 128, :] for i in range(4)], axis=1)

    w1x_a, w1x_b = _stack1(f("Wx1"), f("bx1"))
    w1y_a, w1y_b = _stack1(f("Wy1"), f("by1"))
    cblob = np.zeros((128, 1280), NPBF)
    cblob[:, 0:128] = stack2cols(f("Wx2"))
    cblob[:, 128:256] = stack2cols(wy2f)
    cblob[:, 256:512] = w1x_a
    cblob[0:66, 512:768] = w1x_b
    cblob[:, 768:1024] = w1y_a
    cblob[0:66, 1024:1280] = w1y_b

    bh, bl = _split(f("b_bil").reshape(1, 1))
    fblob = np.zeros((D_EMB, 3), np.float32)
    fblob[:, 0] = f("bx2")
    fblob[:, 1] = by2f
    fblob[0, 2] = np.float32(bh[0, 0])
    fblob[1, 2] = np.float32(bl[0, 0])

    base = {"yts2": yts2, "cblob": cblob, "fblob": fblob}
    in_maps = [
        {**base, "xts2": np.ascontiguousarray(
            xts_full[:, c * 2 * R:(c + 1) * 2 * R])}
        for c in range(N_CORES)
    ]

    if _NC is None:
        _NC = _build()
    res = run_bass_kernel_spmd(_NC, in_maps, list(range(N_CORES)), trace=TRACE)
    LAST_RESULT = res
    scores = np.concatenate(
        [res.results[c]["scores"] for c in range(N_CORES)], axis=0
    )
    return scores, np.zeros((), np.float32)


# revision 32
# speedup vs baseline: 1.2161x; 1.2161x over previous
"""Trainium2 Bass kernel for a bilinear critic:

    xe = relu(x @ Wx1 + bx1) @ Wx2 + bx2          # [B, 32]
    ye = relu(y @ Wy1 + by1) @ Wy2 + by2          # [B, 32]
    scores = (xe @ W_bil) @ ye.T + b_bil[0]       # [B, B]
    returns (scores, 0.0)

B=8192, D_IN=64, D_HID=256, D_EMB=32. 8 NeuronCores, data-parallel over
rows of x (1024 rows/core); the y embedding is computed redundantly on
every core (the MLP is tiny next to the 256 MiB output write, which is
the roofline term). Everything on-chip is kept in TRANSPOSED layout
([feature, batch]) so the contraction dim lands on SBUF partitions with
no on-chip transposes; the host feeds x.T / y.T.

fp32 matmuls run at 1/4 PE rate, so all big GEMMs use an exact 3-term
bf16 hi/lo split (a = ah + al; a@b ~ ah@bh + al@bh + ah@bl, error
~2^-18): the PE streams bf16 at full rate and accumulates in fp32 PSUM.
Splits of inputs/weights are host-precomputed; h's split is computed
on-chip (ACT relu -> hh, DVE (max(psum,0) - hh) -> hl).

Bias folding:
  - by1 enters the layer-1 matmul as two extra contraction rows
    (bf16 hi/lo) against ones-rows of the input pack (exact, and keeps
    relu after the full affine).
  - W_bil and by2 are folded on host into the y-side layer-2:
    ywe = ye @ W_bil^T, so scores = xe_aug^T @ ywe_aug.
  - b_bil enters the scores matmul as two extra hi/lo rows against
    ones-rows on the y side.

Main-loop structure: column chunks of 512 are processed in PAIRS so the
score copies and stores move [128, 1024] at a time (amortized op
overhead; 4 KB contiguous DMA descriptors) and the stationary xst
weights are loaded once per row-block for both chunk matmuls.
"""

import numpy as np
from contextlib import ExitStack

import ml_dtypes
import concourse.bass as bass
import concourse.bacc as bacc
import concourse.tile as tile
from concourse import mybir
from concourse.bass_utils import run_bass_kernel_spmd

B, D_IN, D_HID, D_EMB = 8192, 64, 256, 32
N_CORES = 8
R = B // N_CORES            # 1024 rows of x per core
CH = 512                    # column-chunk width (one PSUM bank of fp32)
NCH = B // CH               # 16 chunks over y's batch dim
RCH = R // CH               # 2 chunks over the core's x rows
KS = 3 * D_EMB + 2          # 98: scores stacked contraction
F32 = mybir.dt.float32
BF16 = mybir.dt.bfloat16
NPBF = ml_dtypes.bfloat16

Relu = mybir.ActivationFunctionType.Relu
Ident = mybir.ActivationFunctionType.Identity
ADD = mybir.AluOpType.add
SUB = mybir.AluOpType.subtract
MAX = mybir.AluOpType.max

TRACE = False               # test.py flips this to profile
LAST_RESULT = None          # BassKernelResults of the last run

_NC = None


def _emit(ctx, tc, t):
    nc = tc.nc
    PS = bass.MemorySpace.PSUM

    consts = ctx.enter_context(tc.tile_pool(name="consts", bufs=1))
    hpool = ctx.enter_context(tc.tile_pool(name="h", bufs=3))
    yepool = ctx.enter_context(tc.tile_pool(name="ye", bufs=4))
    embp = ctx.enter_context(tc.tile_pool(name="emb", bufs=1))
    outp = ctx.enter_context(tc.tile_pool(name="outs", bufs=8))
    ps_h = ctx.enter_context(tc.tile_pool(name="ps_h", bufs=2, space=PS))
    ps_e = ctx.enter_context(tc.tile_pool(name="ps_e", bufs=2, space=PS))
    ps_s = ctx.enter_context(tc.tile_pool(name="ps_s", bufs=2, space=PS))

    # One packed bf16 const blob + one packed f32 bias blob: each DMA
    # instruction costs ~600ns of DGE descriptor emission, so the many
    # small weight/bias loads are consolidated into 2 instructions.
    cb = consts.tile([128, 1280], BF16, tag="cblob")
    nc.sync.dma_start(cb[:], t["cblob"][:])
    fb = consts.tile([D_EMB, 3], F32, tag="fblob")
    nc.sync.dma_start(fb[:], t["fblob"][:])
    w2x = [cb[:, i * 32:(i + 1) * 32] for i in range(4)]
    w2y = [cb[:, 128 + i * 32:128 + (i + 1) * 32] for i in range(4)]
    w1x_a = cb[:, 256:512]          # [w1h; w1l] stacked rows
    w1x_b = cb[0:66, 512:768]       # [w1h; b1h; b1l]
    w1y_a = cb[:, 768:1024]
    w1y_b = cb[0:66, 1024:1280]
    bx2_sb = fb[:, 0:1]
    by2_sb = fb[:, 1:2]
    bb_sb = fb[0:2, 2:3]
    # inputs, host-packed [128, 2n]: per 512-chunk a [128, 1024] block
    # laid out as [[ah; ah] | [al; 1; 1; pad]] pairing with (w1a, w1b)
    x_in = consts.tile([128, 2 * R], BF16, tag="x_in")
    nc.sync.dma_start(x_in[:], t["xts2"][:])
    # whole packed y prefetched into SBUF upfront (3.2 MB) in 4 big
    # DMAs; no per-chunk loads -> stores own the DMA queues.
    y_in = consts.tile([128, 2 * B], BF16, tag="y_in")
    for q in range(4):
        qs = slice(q * B // 2, (q + 1) * B // 2)
        nc.sync.dma_start(y_in[:, qs], t["yts2"][:, qs])

    xst = embp.tile([KS, R], BF16, tag="xst")   # [xh; xl; xh; bh; bl]
    zero2 = consts.tile([2, R], BF16, tag="zero2")
    nc.gpsimd.memset(zero2[:], 0.0)

    def l1_matmuls(ph, w1a, w1b, src, nb, msl):
        """h_pre[128, ch] = w1h.ah + w1l.ah + w1h.al + b1h + b1l."""
        sa = src[:, 2 * nb * CH:(2 * nb + 1) * CH]
        sb = src[0:66, (2 * nb + 1) * CH:(2 * nb + 2) * CH]
        nc.tensor.matmul(ph[:], w1a[:, msl], sa, start=True, stop=False)
        nc.tensor.matmul(ph[:], w1b[:, msl], sb, start=False, stop=True)

    def l2_matmuls(pe_, w2, hh, hl, cs=None):
        """e[32, ch] += w2h.hh + w2l.hh + w2h.hl (6 accumulating mms)."""
        pairs = [(w2[0], hh[0]), (w2[1], hh[1]), (w2[2], hh[0]),
                 (w2[3], hh[1]), (w2[0], hl[0]), (w2[1], hl[1])]
        for i, (w, h) in enumerate(pairs):
            rhs = h[:] if cs is None else h[:, cs]
            nc.tensor.matmul(pe_[:], w[:], rhs,
                             start=(i == 0), stop=(i == len(pairs) - 1))

    # ---- x embedding (8 row-blocks of this core's 1024 rows)
    hhx = [[None, None] for _ in range(RCH)]
    hlx = [[None, None] for _ in range(RCH)]
    for nb in range(RCH):
        for mb in range(2):
            msl = slice(mb * 128, (mb + 1) * 128)
            ph = ps_h.tile([128, CH], F32, tag="ph")
            l1_matmuls(ph, w1x_a, w1x_b, x_in, nb, msl)
            hh = embp.tile([128, CH], BF16, tag=f"hhx{nb}{mb}")
            nc.scalar.activation(hh[:], ph[:], Relu)
            hl = embp.tile([128, CH], BF16, tag=f"hlx{nb}{mb}")
            nc.vector.scalar_tensor_tensor(hl[:], ph[:], 0.0, hh[:], MAX, SUB)
            hhx[nb][mb] = hh
            hlx[nb][mb] = hl

    for nb in range(RCH):
        cs = slice(nb * CH, (nb + 1) * CH)
        pex = ps_e.tile([D_EMB, CH], F32, tag="pe")
        l2_matmuls(pex, w2x, hhx[nb], hlx[nb])
        nc.scalar.activation(xst[0:32, cs], pex[:], Ident, bias=bx2_sb[:])
        nc.vector.scalar_tensor_tensor(
            xst[32:64, cs], pex[:], bx2_sb[:], xst[0:32, cs], ADD, SUB)
        nc.vector.tensor_copy(xst[64:96, cs], xst[0:32, cs])
    # scores-bias rows: xst[96] = bf16_hi(b_bil), xst[97] = bf16_lo(b_bil)
    nc.scalar.activation(xst[96:98, :], zero2[:], Ident, bias=bb_sb[:])

    def y_embed(nb):
        """-> yst tile [98, 512] = [yh; yh; yl; 1; 1] for column chunk nb."""
        hhy, hly = [], []
        for mb in range(2):
            msl = slice(mb * 128, (mb + 1) * 128)
            ph = ps_h.tile([128, CH], F32, tag="ph")
            l1_matmuls(ph, w1y_a, w1y_b, y_in, nb, msl)
            hh = hpool.tile([128, CH], BF16, tag=f"hhy{mb}")
            nc.scalar.activation(hh[:], ph[:], Relu)
            hl = hpool.tile([128, CH], BF16, tag=f"hly{mb}")
            nc.vector.scalar_tensor_tensor(hl[:], ph[:], 0.0, hh[:], MAX, SUB)
            hhy.append(hh)
            hly.append(hl)
        pey = ps_e.tile([D_EMB, CH], F32, tag="pe")
        l2_matmuls(pey, w2y, hhy, hly)
        # xst is [xh; xl; xh; ...], so yst pairs as [yh; yh; yl; 1; 1]
        yst = yepool.tile([KS, CH], BF16, tag="yst")
        nc.scalar.activation(yst[0:32, :], pey[:], Ident, bias=by2_sb[:])
        nc.vector.tensor_copy(yst[32:64, :], yst[0:32, :])
        nc.vector.scalar_tensor_tensor(
            yst[64:96, :], pey[:], by2_sb[:], yst[0:32, :], ADD, SUB)
        nc.gpsimd.memset(yst[96:98, :], 1.0)
        return yst

    # ---- y embedding + scores, chunk PAIRS (1024 output cols at a time)
    for pr in range(NCH // 2):
        nbA, nbB = 2 * pr, 2 * pr + 1
        ystA = y_embed(nbA)
        ystB = y_embed(nbB)
        for mb in range(8):  # row-blocks of 128 within this core's R rows
            ps = ps_s.tile([128, 2 * CH], F32, tag="ps")
            lhs = xst[:, mb * 128:(mb + 1) * 128]
            nc.tensor.matmul(ps[:, 0:CH], lhs, ystA[:], start=True, stop=True)
            nc.tensor.matmul(ps[:, CH:2 * CH], lhs, ystB[:],
                             start=True, stop=True)
            ot = outp.tile([128, 2 * CH], F32, tag="ot")
            # 5:3 ACT:DVE split (DVE also carries the hl/yl arithmetic)
            if mb in (0, 2, 4, 5, 7):
                nc.scalar.activation(ot[:], ps[:], Ident)
            else:
                nc.vector.tensor_copy(ot[:], ps[:])
            nc.sync.dma_start(
                t["scores"][mb * 128:(mb + 1) * 128,
                            nbA * CH:(nbA + 2) * CH], ot[:])


def _build():
    nc = bacc.Bacc(
        "TRN2", target_bir_lowering=False, debug=False, num_devices=N_CORES
    )
    t = {}

    def din(name, shape, dt):
        t[name] = nc.dram_tensor(name, shape, dt, kind="ExternalInput").ap()

    din("xts2", [128, 2 * R], BF16)
    din("yts2", [128, 2 * B], BF16)
    din("cblob", [128, 1280], BF16)
    din("fblob", [D_EMB, 3], F32)
    t["scores"] = nc.dram_tensor("scores", [R, B], F32, kind="ExternalOutput").ap()

    with tile.TileContext(nc) as tc:
        with ExitStack() as ctx:
            _emit(ctx, tc, t)
    nc.compile()
    return nc


def _split(a):
    """f32 array -> (hi, lo) bf16 arrays with a ~= hi + lo (err ~2^-18)."""
    hi = a.astype(NPBF)
    lo = (a - hi.astype(np.float32)).astype(NPBF)
    return hi, lo


def _stack1(w1, b1):
    """Layer-1 weight stacks: a=[w1h; w1l] [128,256], b=[w1h; b1h; b1l] [66,256]."""
    wh, wl = _split(w1)                      # [64, 256]
    bh, bl = _split(b1.reshape(1, -1))       # [1, 256]
    a = np.concatenate([wh, wl], axis=0)
    b = np.concatenate([wh, bh, bl], axis=0)
    return a, b


def _packin(aT):
    """Input pack [128, 2n]: per 512-chunk a [128,1024] block
    [[ah; ah] | [al; 1; 1; 0pad]] pairing with (_stack1.a, _stack1.b)."""
    ah, al = _split(aT)                      # [64, n]
    n = aT.shape[1]
    out = np.zeros((128, 2 * n), NPBF)
    for c in range(n // CH):
        cs = slice(c * CH, (c + 1) * CH)
        blk = slice(2 * c * CH, (2 * c + 1) * CH)
        out[0:64, blk] = ah[:, cs]
        out[64:128, blk] = ah[:, cs]
        blk2 = slice((2 * c + 1) * CH, (2 * c + 2) * CH)
        out[0:64, blk2] = al[:, cs]
        out[64:66, blk2] = np.ones((2, CH), NPBF)
    return out


def kernel(**inputs):
    global _NC, LAST_RESULT
    f = lambda k: np.ascontiguousarray(np.asarray(inputs[k], dtype=np.float32))

    x, y = f("x"), f("y")
    xts_full = _packin(np.ascontiguousarray(x.T))    # [128, 2*8192]
    yts2 = _packin(np.ascontiguousarray(y.T))        # [128, 2*8192]

    # fold W_bil (and by2) into the y-side layer 2: ywe = ye @ W_bil^T
    Wb = f("W_bil").astype(np.float64)
    wy2f = (f("Wy2").astype(np.float64) @ Wb.T).astype(np.float32)
    by2f = (f("by2").astype(np.float64) @ Wb.T).astype(np.float32)

    def stack2cols(w2):
        """[256,32] -> [128, 128]: 4 side-by-side K-chunks of [w2h; w2l]."""
        wh, wl = _split(w2)
        s = np.concatenate([wh, wl], axis=0)     # [512, 32]
        return np.concatenate(
            [s[i * 128:(i + 1) * 128, :] for i in range(4)], axis=1)

    w1x_a, w1x_b = _stack1(f("Wx1"), f("bx1"))
    w1y_a, w1y_b = _stack1(f("Wy1"), f("by1"))
    cblob = np.zeros((128, 1280), NPBF)
    cblob[:, 0:128] = stack2cols(f("Wx2"))
    cblob[:, 128:256] = stack2cols(wy2f)
    cblob[:, 256:512] = w1x_a
    cblob[0:66, 512:768] = w1x_b
    cblob[:, 768:1024] = w1y_a
    cblob[0:66, 1024:1280] = w1y_b

    bh, bl = _split(f("b_bil").reshape(1, 1))
    fblob = np.zeros((D_EMB, 3), np.float32)
    fblob[:, 0] = f("bx2")
    fblob[:, 1] = by2f
    fblob[0, 2] = np.float32(bh[0, 0])
    fblob[1, 2] = np.float32(bl[0, 0])

    base = {"yts2": yts2, "cblob": cblob, "fblob": fblob}
    in_maps = [
        {**base, "xts2": np.ascontiguousarray(
            xts_full[:, c * 2 * R:(c + 1) * 2 * R])}
        for c in range(N_CORES)
    ]

    if _NC is None:
        _NC = _build()
    res = run_bass_kernel_spmd(_NC, in_maps, list(range(N_CORES)), trace=TRACE)
    LAST_RESULT = res
    scores = np.concatenate(
        [res.results[c]["scores"] for c in range(N_CORES)], axis=0
    )
    return scores, np.zeros((), np.float32)


# revision 42
# speedup vs baseline: 1.2436x; 1.0226x over previous
"""Trainium2 Bass kernel for a bilinear critic:

    xe = relu(x @ Wx1 + bx1) @ Wx2 + bx2          # [B, 32]
    ye = relu(y @ Wy1 + by1) @ Wy2 + by2          # [B, 32]
    scores = (xe @ W_bil) @ ye.T + b_bil[0]       # [B, B]
    returns (scores, 0.0)

B=8192, D_IN=64, D_HID=256, D_EMB=32. 8 NeuronCores, data-parallel over
rows of x (1024 rows/core); the y embedding is computed redundantly on
every core (the MLP is tiny next to the 256 MiB output write, which is
the roofline term). Everything on-chip is kept in TRANSPOSED layout
([feature, batch]) so the contraction dim lands on SBUF partitions with
no on-chip transposes; the host feeds x.T / y.T.

fp32 matmuls run at 1/4 PE rate, so all big GEMMs use an exact 3-term
bf16 hi/lo split (a = ah + al; a@b ~ ah@bh + al@bh + ah@bl, error
~2^-18): the PE streams bf16 at full rate and accumulates in fp32 PSUM.
Splits of inputs/weights are host-precomputed; h's split is computed
on-chip (ACT relu -> hh, DVE (max(psum,0) - hh) -> hl).

Bias folding:
  - by1 enters the layer-1 matmul as two extra contraction rows
    (bf16 hi/lo) against ones-rows of the input pack (exact, and keeps
    relu after the full affine).
  - W_bil and by2 are folded on host into the y-side layer-2:
    ywe = ye @ W_bil^T, so scores = xe_aug^T @ ywe_aug.
  - b_bil enters the scores matmul as two extra hi/lo rows against
    ones-rows on the y side.

Main-loop structure: column chunks of 512 are processed in PAIRS so the
score copies and stores move [128, 1024] at a time (amortized op
overhead; 4 KB contiguous DMA descriptors) and the stationary xst
weights are loaded once per row-block for both chunk matmuls.
"""

import numpy as np
from contextlib import ExitStack

import ml_dtypes
import concourse.bass as bass
import concourse.bacc as bacc
import concourse.tile as tile
from concourse import mybir
from concourse.bass_utils import run_bass_kernel_spmd

B, D_IN, D_HID, D_EMB = 8192, 64, 256, 32
N_CORES = 8
R = B // N_CORES            # 1024 rows of x per core
CH = 512                    # column-chunk width (one PSUM bank of fp32)
NCH = B // CH               # 16 chunks over y's batch dim
RCH = R // CH               # 2 chunks over the core's x rows
KS = 3 * D_EMB + 2          # 98: scores stacked contraction
F32 = mybir.dt.float32
BF16 = mybir.dt.bfloat16
NPBF = ml_dtypes.bfloat16

Relu = mybir.ActivationFunctionType.Relu
Ident = mybir.ActivationFunctionType.Identity
ADD = mybir.AluOpType.add
SUB = mybir.AluOpType.subtract
MAX = mybir.AluOpType.max

TRACE = False               # test.py flips this to profile
LAST_RESULT = None          # BassKernelResults of the last run

_NC = None


def _emit(ctx, tc, t):
    nc = tc.nc
    PS = bass.MemorySpace.PSUM

    consts = ctx.enter_context(tc.tile_pool(name="consts", bufs=1))
    hpool = ctx.enter_context(tc.tile_pool(name="h", bufs=4))
    yepool = ctx.enter_context(tc.tile_pool(name="ye", bufs=6))
    embp = ctx.enter_context(tc.tile_pool(name="emb", bufs=1))
    outp = ctx.enter_context(tc.tile_pool(name="outs", bufs=10))
    ps_h = ctx.enter_context(tc.tile_pool(name="ps_h", bufs=2, space=PS))
    ps_e = ctx.enter_context(tc.tile_pool(name="ps_e", bufs=2, space=PS))
    ps_s = ctx.enter_context(tc.tile_pool(name="ps_s", bufs=2, space=PS))

    # One packed bf16 const blob + one packed f32 bias blob: each DMA
    # instruction costs ~600ns of DGE descriptor emission, so the many
    # small weight/bias loads are consolidated into 2 instructions.
    cb = consts.tile([128, 1280], BF16, tag="cblob")
    nc.sync.dma_start(cb[:], t["cblob"][:])
    fb = consts.tile([D_EMB, 3], F32, tag="fblob")
    nc.sync.dma_start(fb[:], t["fblob"][:])
    w2x = [cb[:, i * 32:(i + 1) * 32] for i in range(4)]
    w2y = [cb[:, 128 + i * 32:128 + (i + 1) * 32] for i in range(4)]
    w1x_a = cb[:, 256:512]          # [w1h; w1l] stacked rows
    w1x_b = cb[0:66, 512:768]       # [w1h; b1h; b1l]
    w1y_a = cb[:, 768:1024]
    w1y_b = cb[0:66, 1024:1280]
    bx2_sb = fb[:, 0:1]
    by2_sb = fb[:, 1:2]
    bb_sb = fb[0:2, 2:3]
    # inputs, host-packed [128, 2n]: per 512-chunk a [128, 1024] block
    # laid out as [[ah; ah] | [al; 1; 1; pad]] pairing with (w1a, w1b)
    x_in = consts.tile([128, 2 * R], BF16, tag="x_in")
    nc.sync.dma_start(x_in[:], t["xts2"][:])
    # whole packed y prefetched into SBUF upfront (3.2 MB) in 4 big
    # DMAs; no per-chunk loads -> stores own the DMA queues.
    y_in = consts.tile([128, 2 * B], BF16, tag="y_in")
    for q in range(4):
        qs = slice(q * B // 2, (q + 1) * B // 2)
        nc.sync.dma_start(y_in[:, qs], t["yts2"][:, qs])

    xst = embp.tile([KS, R], BF16, tag="xst")   # [xh; xl; xh; bh; bl]
    zero2 = consts.tile([2, R], BF16, tag="zero2")
    nc.gpsimd.memset(zero2[:], 0.0)
    # scores-bias rows first (high priority -- every score mm reads them):
    # xst[96] = bf16_hi(b_bil), xst[97] = bf16_lo(b_bil)
    nc.scalar.activation(xst[96:98, :], zero2[:], Ident, bias=bb_sb[:])

    # PE warm-up: ~4us of throwaway matmuls during the load ramp so the
    # HAM clock gate reaches 8/8 (2.4 GHz) before the real chain starts.
    warm = ps_h.tile([128, CH], F32, tag="ph")
    for _ in range(10):
        nc.tensor.matmul(warm[:], zero2[:, 0:128], zero2[:, 0:CH],
                         start=True, stop=True)

    def l1_matmuls(ph, w1a, w1b, src, nb, msl):
        """h_pre[128, ch] = w1h.ah + w1l.ah + w1h.al + b1h + b1l."""
        sa = src[:, 2 * nb * CH:(2 * nb + 1) * CH]
        sb = src[0:66, (2 * nb + 1) * CH:(2 * nb + 2) * CH]
        nc.tensor.matmul(ph[:], w1a[:, msl], sa, start=True, stop=False)
        nc.tensor.matmul(ph[:], w1b[:, msl], sb, start=False, stop=True)

    def l2_matmuls(pe_, w2, hh, hl, cs=None):
        """e[32, ch] += w2h.hh + w2l.hh + w2h.hl (6 accumulating mms)."""
        pairs = [(w2[0], hh[0]), (w2[1], hh[1]), (w2[2], hh[0]),
                 (w2[3], hh[1]), (w2[0], hl[0]), (w2[1], hl[1])]
        for i, (w, h) in enumerate(pairs):
            rhs = h[:] if cs is None else h[:, cs]
            nc.tensor.matmul(pe_[:], w[:], rhs,
                             start=(i == 0), stop=(i == len(pairs) - 1))

    def x_embed(nb):
        """Fill xst columns for x-chunk nb (row-blocks 4nb..4nb+3)."""
        cs = slice(nb * CH, (nb + 1) * CH)
        hh2, hl2 = [], []
        for mb in range(2):
            msl = slice(mb * 128, (mb + 1) * 128)
            ph = ps_h.tile([128, CH], F32, tag="ph")
            l1_matmuls(ph, w1x_a, w1x_b, x_in, nb, msl)
            hh = embp.tile([128, CH], BF16, tag=f"hhx{nb}{mb}")
            nc.scalar.activation(hh[:], ph[:], Relu)
            hl = embp.tile([128, CH], BF16, tag=f"hlx{nb}{mb}")
            nc.vector.scalar_tensor_tensor(hl[:], ph[:], 0.0, hh[:], MAX, SUB)
            hh2.append(hh)
            hl2.append(hl)
        pex = ps_e.tile([D_EMB, CH], F32, tag="pe")
        l2_matmuls(pex, w2x, hh2, hl2)
        nc.scalar.activation(xst[0:32, cs], pex[:], Ident, bias=bx2_sb[:])
        nc.vector.scalar_tensor_tensor(
            xst[32:64, cs], pex[:], bx2_sb[:], xst[0:32, cs], ADD, SUB)
        nc.vector.tensor_copy(xst[64:96, cs], xst[0:32, cs])

    def y_embed(nb):
        """-> yst tile [98, 512] = [yh; yh; yl; 1; 1] for column chunk nb."""
        hhy, hly = [], []
        for mb in range(2):
            msl = slice(mb * 128, (mb + 1) * 128)
            ph = ps_h.tile([128, CH], F32, tag="ph")
            l1_matmuls(ph, w1y_a, w1y_b, y_in, nb, msl)
            hh = hpool.tile([128, CH], BF16, tag=f"hhy{mb}")
            nc.scalar.activation(hh[:], ph[:], Relu)
            hl = hpool.tile([128, CH], BF16, tag=f"hly{mb}")
            nc.vector.scalar_tensor_tensor(hl[:], ph[:], 0.0, hh[:], MAX, SUB)
            hhy.append(hh)
            hly.append(hl)
        pey = ps_e.tile([D_EMB, CH], F32, tag="pe")
        l2_matmuls(pey, w2y, hhy, hly)
        # xst is [xh; xl; xh; ...], so yst pairs as [yh; yh; yl; 1; 1]
        yst = yepool.tile([KS, CH], BF16, tag="yst")
        nc.scalar.activation(yst[0:32, :], pey[:], Ident, bias=by2_sb[:])
        nc.vector.tensor_copy(yst[32:64, :], yst[0:32, :])
        nc.vector.scalar_tensor_tensor(
            yst[64:96, :], pey[:], by2_sb[:], yst[0:32, :], ADD, SUB)
        nc.gpsimd.memset(yst[96:98, :], 1.0)
        return yst

    # ---- emission order compresses the ramp: x chunk 0 completes, then
    # pair 0's y embeds (so row-blocks 0-3 of pair 0 can store while x
    # chunk 1 is still in flight), then x chunk 1, then the pair stream.
    x_embed(0)
    for pr in range(NCH // 2):
        nbA, nbB = 2 * pr, 2 * pr + 1
        ystA = y_embed(nbA)
        ystB = y_embed(nbB)
        if pr == 0:
            x_embed(1)
        for mb in range(8):  # row-blocks of 128 within this core's R rows
            ps = ps_s.tile([128, 2 * CH], F32, tag="ps")
            lhs = xst[:, mb * 128:(mb + 1) * 128]
            nc.tensor.matmul(ps[:, 0:CH], lhs, ystA[:], start=True, stop=True)
            nc.tensor.matmul(ps[:, CH:2 * CH], lhs, ystB[:],
                             start=True, stop=True)
            ot = outp.tile([128, 2 * CH], F32, tag="ot")
            # 5:3 ACT:DVE split (DVE also carries the hl/yl arithmetic)
            if mb in (0, 2, 4, 5, 7):
                nc.scalar.activation(ot[:], ps[:], Ident)
            else:
                nc.vector.tensor_copy(ot[:], ps[:])
            nc.sync.dma_start(
                t["scores"][mb * 128:(mb + 1) * 128,
                            nbA * CH:(nbA + 2) * CH], ot[:])


def _build():
    nc = bacc.Bacc(
        "TRN2", target_bir_lowering=False, debug=False, num_devices=N_CORES
    )
    t = {}

    def din(name, shape, dt):
        t[name] = nc.dram_tensor(name, shape, dt, kind="ExternalInput").ap()

    din("xts2", [128, 2 * R], BF16)
    din("yts2", [128, 2 * B], BF16)
    din("cblob", [128, 1280], BF16)
    din("fblob", [D_EMB, 3], F32)
    t["scores"] = nc.dram_tensor("scores", [R, B], F32, kind="ExternalOutput").ap()

    with tile.TileContext(nc) as tc:
        with ExitStack() as ctx:
            _emit(ctx, tc, t)
    nc.compile()
    return nc


def _split(a):
    """f32 array -> (hi, lo) bf16 arrays with a ~= hi + lo (err ~2^-18)."""
    hi = a.astype(NPBF)
    lo = (a - hi.astype(np.float32)).astype(NPBF)
    return hi, lo


def _stack1(w1, b1):
    """Layer-1 weight stacks: a=[w1h; w1l] [128,256], b=[w1h; b1h; b1l] [66,256]."""
    wh, wl = _split(w1)                      # [64, 256]
    bh, bl = _split(b1.reshape(1, -1))       # [1, 256]
    a = np.concatenate([wh, wl], axis=0)
    b = np.concatenate([wh, bh, bl], axis=0)
    return a, b


def _packin(aT):
    """Input pack [128, 2n]: per 512-chunk a [128,1024] block
    [[ah; ah] | [al; 1; 1; 0pad]] pairing with (_stack1.a, _stack1.b)."""
    ah, al = _split(aT)                      # [64, n]
    n = aT.shape[1]
    out = np.zeros((128, 2 * n), NPBF)
    for c in range(n // CH):
        cs = slice(c * CH, (c + 1) * CH)
        blk = slice(2 * c * CH, (2 * c + 1) * CH)
        out[0:64, blk] = ah[:, cs]
        out[64:128, blk] = ah[:, cs]
        blk2 = slice((2 * c + 1) * CH, (2 * c + 2) * CH)
        out[0:64, blk2] = al[:, cs]
        out[64:66, blk2] = np.ones((2, CH), NPBF)
    return out


def kernel(**inputs):
    global _NC, LAST_RESULT
    f = lambda k: np.ascontiguousarray(np.asarray(inputs[k], dtype=np.float32))

    x, y = f("x"), f("y")
    xts_full = _packin(np.ascontiguousarray(x.T))    # [128, 2*8192]
    yts2 = _packin(np.ascontiguousarray(y.T))        # [128, 2*8192]

    # fold W_bil (and by2) into the y-side layer 2: ywe = ye @ W_bil^T
    Wb = f("W_bil").astype(np.float64)
    wy2f = (f("Wy2").astype(np.float64) @ Wb.T).astype(np.float32)
    by2f = (f("by2").astype(np.float64) @ Wb.T).astype(np.float32)

    def stack2cols(w2):
        """[256,32] -> [128, 128]: 4 side-by-side K-chunks of [w2h; w2l]."""
        wh, wl = _split(w2)
        s = np.concatenate([wh, wl], axis=0)     # [512, 32]
        return np.concatenate(
            [s[i * 128:(i + 1) * 128, :] for i in range(4)], axis=1)

    w1x_a, w1x_b = _stack1(f("Wx1"), f("bx1"))
    w1y_a, w1y_b = _stack1(f("Wy1"), f("by1"))
    cblob = np.zeros((128, 1280), NPBF)
    cblob[:, 0:128] = stack2cols(f("Wx2"))
    cblob[:, 128:256] = stack2cols(wy2f)
    cblob[:, 256:512] = w1x_a
    cblob[0:66, 512:768] = w1x_b
    cblob[:, 768:1024] = w1y_a
    cblob[0:66, 1024:1280] = w1y_b

    bh, bl = _split(f("b_bil").reshape(1, 1))
    fblob = np.zeros((D_EMB, 3), np.float32)
    fblob[:, 0] = f("bx2")
    fblob[:, 1] = by2f
    fblob[0, 2] = np.float32(bh[0, 0])
    fblob[1, 2] = np.float32(bl[0, 0])

    base = {"yts2": yts2, "cblob": cblob, "fblob": fblob}
    in_maps = [
        {**base, "xts2": np.ascontiguousarray(
            xts_full[:, c * 2 * R:(c + 1) * 2 * R])}
        for c in range(N_CORES)
    ]

    if _NC is None:
        _NC = _build()
    res = run_bass_kernel_spmd(_NC, in_maps, list(range(N_CORES)), trace=TRACE)
    LAST_RESULT = res
    scores = np.concatenate(
        [res.results[c]["scores"] for c in range(N_CORES)], axis=0
    )
    return scores, np.zeros((), np.float32)
